# revision 3
# baseline (speedup 1.0000x reference)
"""Trainium2 Bass kernel for nn_Attention_81655918231876.

RoPE attention with positional bias, 8 heads / dim_head 64, b=2, n=2048, dim=512.
Sharding: head-parallel across 8 cores. Core h computes head h for BOTH batches
and emits a partial output y_h = softmax(q_h k_h^T + bias_h) v_h @ w_out[h-slice].
The host sums the 8 partials (each core's partial is a full [2, 2048, 512] array).

Device-side layout (per core):
  - Projections computed in transposed form qT/kT [64, n] via matmuls with the
    weight block as stationary operand and x^T as the moving operand.
  - RoPE is folded into the projection: extra "rotated" weight columns
    W_rot[:, 2i] = -W[:, 2i+1], W_rot[:, 2i+1] = W[:, 2i] give rot_half(q)^T
    directly, then qT = cos*qT_raw + sin*qT_rot elementwise (cos/sin tables
    are host-precomputed in [d, n] layout).
  - Scores per 128-row i-block accumulate q^T-block x k^T in PSUM; the
    positional bias is added by a second matmul with an identity stationary
    operand (PSUM accumulate), so no extra vector pass is needed.
  - exp runs on the scalar engine reading PSUM directly, with accum_out
    producing row sums for free. Softmax max-subtraction is skipped: scores
    are O(10) and exp is exact fp32-safe there, matching softmax exactly.
  - P is transposed 128x128-blockwise on the PE; P^T feeds the O^T = V^T P^T
    accumulation. The 1/rowsum normalization is folded into the final y
    projection as a per-partition tensor_scalar multiply.
  - Matmul operands use float32r (full-rate fp32 PE mode, free dim >= 256).
"""

import numpy as np
import sys

sys.path.insert(0, "/opt/trn_rl_repo")

HEADS = 8
DIM_HEAD = 64
ROPE_THETA = 10000.0
B, N, DIM = 2, 2048, 512
NB = N // 128  # 16 i-blocks

_compiled = None


def _build():
    import concourse.bass as bass
    import concourse.tile as tile
    from concourse import bacc, mybir

    f32 = mybir.dt.float32
    f32r = mybir.dt.float32r
    Exp = mybir.ActivationFunctionType.Exp

    nc = bacc.Bacc(None, target_bir_lowering=False, debug=False)
    xt = nc.dram_tensor("xt", [DIM, 2 * N], f32r, kind="ExternalInput")
    wall = nc.dram_tensor("wall", [DIM, 256], f32r, kind="ExternalInput")
    wv = nc.dram_tensor("wv", [DIM, 64], f32r, kind="ExternalInput")
    cs2 = nc.dram_tensor("cs2", [128, N], f32, kind="ExternalInput")
    bias = nc.dram_tensor("bias", [N, N], f32r, kind="ExternalInput")
    wo = nc.dram_tensor("wo", [64, DIM], f32r, kind="ExternalInput")
    idf = nc.dram_tensor("idf", [128, 128], f32, kind="ExternalInput")
    idr = nc.dram_tensor("idr", [128, 128], f32r, kind="ExternalInput")
    out = nc.dram_tensor("out", [B, N, DIM], f32, kind="ExternalOutput")

    with tile.TileContext(nc) as tc:
        with (
            tc.tile_pool(name="singles", bufs=1) as singles,
            tc.tile_pool(name="xtp", bufs=4) as xtp,
            tc.tile_pool(name="biasp", bufs=2) as biasp,
            tc.tile_pool(name="pp", bufs=2) as pp,
            tc.tile_pool(name="ptp", bufs=18) as ptp,
            tc.tile_pool(name="t1p", bufs=3) as t1p,
            tc.tile_pool(name="yp", bufs=3) as yp,
            tc.tile_pool(name="smp", bufs=4) as smp,
            tc.tile_pool(name="psA", bufs=2, space="PSUM") as psA,
            tc.tile_pool(name="psB", bufs=2, space="PSUM") as psB,
            tc.tile_pool(name="psC", bufs=2, space="PSUM") as psC,
        ):
            # constants
            wl_sb = [singles.tile([128, 256], f32r, tag=f"wl{k}", name=f"wl_sb{k}") for k in range(4)]
            wv_sb = [singles.tile([128, 64], f32r, tag=f"wv{k}", name=f"wv_sb{k}") for k in range(4)]
            for k in range(4):
                nc.sync.dma_start(out=wl_sb[k], in_=wall[128 * k:128 * (k + 1), :])
                nc.sync.dma_start(out=wv_sb[k], in_=wv[128 * k:128 * (k + 1), :])
            cs_sb = singles.tile([128, N], f32, tag="cs")
            nc.sync.dma_start(out=cs_sb, in_=cs2[:, :])
            idf_sb = singles.tile([128, 128], f32, tag="idf")
            nc.sync.dma_start(out=idf_sb, in_=idf[:, :])
            idr_sb = singles.tile([128, 128], f32r, tag="idr")
            nc.sync.dma_start(out=idr_sb, in_=idr[:, :])
            wo_sb = singles.tile([64, DIM], f32r, tag="wo")
            nc.sync.dma_start(out=wo_sb, in_=wo[:, :])

            for b in range(B):
                # ---- projection phase ----
                xb = [xtp.tile([128, N], f32r, tag="xt", name=f"xb{b}_{_k}") for _k in range(4)]
                for k in range(4):
                    nc.sync.dma_start(
                        out=xb[k], in_=xt[128 * k:128 * (k + 1), b * N:(b + 1) * N]
                    )
                qf = singles.tile([64, N], f32r, tag=f"qf{b}")
                kf = singles.tile([64, N], f32r, tag=f"kf{b}")
                for mt, dst in ((0, qf), (1, kf)):
                    for nch in range(4):
                        ps = psA.tile([128, 512], f32, tag="s")
                        for k in range(4):
                            nc.tensor.matmul(
                                ps,
                                wl_sb[k][:, 128 * mt:128 * (mt + 1)],
                                xb[k][:, 512 * nch:512 * (nch + 1)],
                                start=(k == 0),
                                stop=(k == 3),
                            )
                        sl = slice(512 * nch, 512 * (nch + 1))
                        t1 = t1p.tile([64, 512], f32, tag="t1")
                        t2 = t1p.tile([64, 512], f32, tag="t2")
                        nc.vector.tensor_mul(t1, ps[0:64, :], cs_sb[0:64, sl])
                        nc.vector.tensor_mul(t2, ps[64:128, :], cs_sb[64:128, sl])
                        nc.vector.tensor_add(dst[:, sl], t1, t2)
                vsb = singles.tile([128, 16 * 64], f32r, tag=f"v{b}")
                for it in range(16):
                    ps = psA.tile([128, 64], f32, tag="s")
                    for k in range(4):
                        nc.tensor.matmul(
                            ps,
                            xb[k][:, 128 * it:128 * (it + 1)],
                            wv_sb[k],
                            start=(k == 0),
                            stop=(k == 3),
                        )
                    nc.vector.tensor_copy(vsb[:, 64 * it:64 * (it + 1)], ps)

                # ---- attention phase ----
                sums_u = singles.tile([128, NB], f32, tag=f"sums{b}")
                rinv_u = singles.tile([128, NB], f32, tag=f"rinv{b}")
                otsb = singles.tile([64, N], f32r, tag=f"ot{b}")
                for isup in range(4):
                    ptile = [ptp.tile([128, 512], f32r, tag="pt", name=f"ptile_{b}_{isup}_{_j}") for _j in range(16)]
                    for ib4 in range(4):
                        ib = isup * 4 + ib4
                        i0 = ib * 128
                        bias_sb = biasp.tile([128, N], f32r, tag="bias")
                        nc.sync.dma_start(out=bias_sb, in_=bias[i0:i0 + 128, :])
                        p_sb = pp.tile([128, N], f32, tag="p")
                        sm_h = smp.tile([128, 2], f32, tag="sm")
                        for half in range(2):
                            s_ps = psA.tile([128, 1024], f32, tag="s")
                            for nch in range(2):
                                j0 = half * 1024 + nch * 512
                                sl = slice(512 * nch, 512 * (nch + 1))
                                nc.tensor.matmul(
                                    s_ps[:, sl],
                                    qf[:, i0:i0 + 128],
                                    kf[:, j0:j0 + 512],
                                    start=True,
                                    stop=False,
                                )
                                nc.tensor.matmul(
                                    s_ps[:, sl],
                                    idr_sb,
                                    bias_sb[:, j0:j0 + 512],
                                    start=False,
                                    stop=True,
                                )
                            nc.scalar.activation(
                                p_sb[:, 1024 * half:1024 * (half + 1)],
                                s_ps,
                                Exp,
                                accum_out=sm_h[:, half:half + 1],
                            )
                        nc.vector.tensor_add(
                            sums_u[:, ib:ib + 1], sm_h[:, 0:1], sm_h[:, 1:2]
                        )
                        nc.vector.reciprocal(
                            rinv_u[:, ib:ib + 1], sums_u[:, ib:ib + 1]
                        )
                        for jt in range(16):
                            pt_ps = psB.tile([128, 128], f32, tag="pt")
                            nc.tensor.transpose(
                                pt_ps, p_sb[:, 128 * jt:128 * (jt + 1)], idf_sb
                            )
                            nc.vector.tensor_copy(
                                ptile[jt][:, 128 * ib4:128 * (ib4 + 1)], pt_ps
                            )
                    ot_ps = psC.tile([64, 512], f32, tag="o")
                    for jt in range(16):
                        nc.tensor.matmul(
                            ot_ps,
                            vsb[:, 64 * jt:64 * (jt + 1)],
                            ptile[jt],
                            start=(jt == 0),
                            stop=(jt == 15),
                        )
                    nc.vector.tensor_copy(otsb[:, 512 * isup:512 * (isup + 1)], ot_ps)

                # ---- output projection ----
                for ib in range(NB):
                    y_ps = psC.tile([128, 512], f32, tag="o")
                    nc.tensor.matmul(
                        y_ps, otsb[:, 128 * ib:128 * (ib + 1)], wo_sb,
                        start=True, stop=True,
                    )
                    y_sb = yp.tile([128, 512], f32, tag="y")
                    nc.vector.tensor_scalar_mul(y_sb, y_ps, rinv_u[:, ib:ib + 1])
                    nc.sync.dma_start(
                        out=out[b, 128 * ib:128 * (ib + 1), :], in_=y_sb
                    )

    nc.compile()
    return nc


def _host_inputs(x, pos_bias, w_qkv, w_out):
    """Build the per-core input maps (head-parallel sharding)."""
    x = np.asarray(x, dtype=np.float32)
    pos_bias = np.ascontiguousarray(np.asarray(pos_bias, dtype=np.float32))
    w_qkv = np.asarray(w_qkv, dtype=np.float32)
    w_out = np.asarray(w_out, dtype=np.float32)
    hidden = HEADS * DIM_HEAD

    xt = np.ascontiguousarray(
        np.concatenate([x[0].T, x[1].T], axis=1)
    )  # [512, 4096]

    inv_freq = 1.0 / (ROPE_THETA ** (np.arange(0, DIM_HEAD, 2, dtype=np.float64) / DIM_HEAD))
    freqs = np.arange(N, dtype=np.float64)[:, None] * inv_freq[None, :]  # [n, 32]
    freqs = np.repeat(freqs, 2, axis=-1)  # [n, 64]
    cosT = np.cos(freqs).T.astype(np.float32)  # [64, n]
    sinT = np.sin(freqs).T.astype(np.float32)
    cs2 = np.ascontiguousarray(np.concatenate([cosT, sinT], axis=0))  # [128, n]

    def rot_cols(w):
        wr = np.empty_like(w)
        wr[:, 0::2] = -w[:, 1::2]
        wr[:, 1::2] = w[:, 0::2]
        return wr

    scale = DIM_HEAD ** -0.5
    ident = np.eye(128, dtype=np.float32)
    in_maps = []
    for h in range(HEADS):
        wq = w_qkv[:, h * 64:(h + 1) * 64] * scale
        wk = w_qkv[:, hidden + h * 64:hidden + (h + 1) * 64]
        wvh = w_qkv[:, 2 * hidden + h * 64:2 * hidden + (h + 1) * 64]
        wall = np.ascontiguousarray(
            np.concatenate([wq, rot_cols(wq), wk, rot_cols(wk)], axis=1)
        )  # [512, 256]
        in_maps.append({
            "xt": xt,
            "wall": wall,
            "wv": np.ascontiguousarray(wvh),
            "cs2": cs2,
            "bias": np.ascontiguousarray(pos_bias[h]),
            "wo": np.ascontiguousarray(w_out[h * 64:(h + 1) * 64, :]),
            "idf": ident,
            "idr": ident,
        })
    return in_maps


def kernel(x, pos_bias, w_qkv, w_out, _want_trace=False):
    global _compiled
    from concourse.bass_utils import run_bass_kernel_spmd

    if _compiled is None:
        _compiled = _build()
    in_maps = _host_inputs(x, pos_bias, w_qkv, w_out)
    res = run_bass_kernel_spmd(
        _compiled, in_maps, core_ids=list(range(HEADS)), trace=_want_trace
    )
    y = np.zeros((B, N, DIM), dtype=np.float32)
    for r in res.results:
        y += r["out"]
    if _want_trace:
        kernel._last_results = res
    return y
